# revision 11
# baseline (speedup 1.0000x reference)
"""Trainium2 Bass kernel for nn_FCGF_RP_AVG (topk masking + masked mean + L2 norm).

Per segment b of 64 (L=50000 points, D=32 feats):
  att = x @ w ; mask = top-1024 of att ; res = mask @ x ; out = res/||res||

8 segments per core (data parallel over 8 NeuronCores).

v2 design (PE-centric, bf16 streams, single-pass phases):
  Layout: L padded to 50176 = 16*3136. OLD layout: partition p = 16*s + sub
  holds segment s's points [sub*3136, (sub+1)*3136). QUAD layout for phase A:
  partition = 32*q + d (quad q = segments {2q, 2q+1}), free = point.

  Phase A: stream x_T (quad layout, bf16, contiguous 28KB lines), att via PE
  matmul (lhsT = wq [128,4]) -> PSUM [4, 512] chunks; stage groups of 4 banks
  to SBUF via scalar/gpsimd/vector copies; SBUF->SBUF DMA redistributes to
  att [128, 3136] f32 in OLD layout.

  Phase B: threshold bisection, 13 iters: fused DVE indicator+count pass
  (scalar_tensor_tensor accum_out), per-segment count replication via a
  single [128,128] block-ones matmul, predicated lo/hi update.

  Phase C: stream x (OLD layout, bf16): hybrid masked sum:
    - PE path: per 16-point chunk lhsT[p, (j,s)] = mask[p,j]*blk[p,s] (built
      on DVE/gpsimd), matmul vs x chunk accumulating diag blocks in one PSUM
      bank; diagonal extracted with 16 tiny adds.
    - dense path: gpsimd in-place mask-mult + DVE strided reduce for the
      remaining j-range of each stream tile.
  Final: per-segment sums -> L2 normalize -> out [8, 32].
"""

import numpy as np

B = 64
L = 50000
D = 32
TOPK = 1024
NCORES = 8
SEG = B // NCORES          # 8 segments per core
P = 128
LP = 50176                 # padded segment length = 16 * 3136
JP = LP // 16              # 3136 points per partition (OLD layout)
NPQ = 2 * LP               # 100352 points per quad (phase A free dim)
NCOL = NPQ                 # columns per partition in both DRAM layouts

CH_A = 512                 # phase A matmul cols
GRP_A = 4                  # chunks per PSUM group
NG_A = NPQ // (CH_A * GRP_A)   # 49 groups
ST_A = 14336               # stream tile cols (7 groups)
NST_A = NCOL // ST_A       # 7 stream DMAs

JT_C = 448                 # j per phase C stream tile
NST_C = JP // JT_C         # 7 stream tiles
JC = 16                    # j per PE chunk (lhsT 128 cols)
NPE_C = 19                 # PE chunks per tile (304 j); dense = 144 j
JDENSE = JT_C - NPE_C * JC # 144

NITER = 13

_CACHE = {}


def _hoist_sync_waits(nc):
    """Move per-instruction semaphore waits onto standalone EventSemaphore
    instructions (walrus rejects instructions whose ISA struct lacks enough
    sync-wait slots)."""
    import bass_rust
    from concourse import mybir

    n = 0
    for bbw in nc.bb_map.values():
        bb = bbw.bb
        new = []
        for inst in bb.instructions:
            si = inst.sync_info
            if si is not None and si.on_wait and not isinstance(
                inst, bass_rust.InstEventSemaphore
            ):
                for k, w in enumerate(si.on_wait):
                    ev = mybir.InstEventSemaphore(
                        name=f"{inst.name}-w{k}", ins=[], outs=[],
                        sync_info=mybir.SyncInfo(on_update=[], on_wait=[w]))
                    ev.engine = inst.engine
                    new.append(ev)
                    n += 1
                inst.sync_info = mybir.SyncInfo(
                    on_update=list(si.on_update), on_wait=[])
            new.append(inst)
        bb.instructions = new
    return n


def _redist_pieces():
    """Per phase-A group: list of (src_off, dst_part_start, j0, length)."""
    out = []
    for g in range(NG_A):
        start = g * CH_A * GRP_A
        end = start + CH_A * GRP_A
        pieces = []
        a = start
        while a < end:
            b = min(end, (a // JP + 1) * JP)
            bseg = a // LP
            off = a % LP
            sub = off // JP
            j0 = off % JP
            pieces.append((a - start, 16 * bseg + sub, j0, b - a))
            a = b
        out.append(pieces)
    return out


def _build():
    import concourse.bass as bass
    import concourse.tile as tile
    from concourse import mybir

    nc = bass.Bass()
    f32 = mybir.dt.float32
    bf16 = mybir.dt.bfloat16
    Alu = mybir.AluOpType
    Act = mybir.ActivationFunctionType

    xt_d = nc.dram_tensor("xt", [P, NCOL], bf16, kind="ExternalInput")
    xo_d = nc.dram_tensor("xo", [P, NCOL], bf16, kind="ExternalInput")
    wq_d = nc.dram_tensor("wq", [P, 4], bf16, kind="ExternalInput")
    blkb_d = nc.dram_tensor("blkb", [P, SEG], bf16, kind="ExternalInput")
    blkf_d = nc.dram_tensor("blkf", [P, SEG], f32, kind="ExternalInput")
    blkseg_d = nc.dram_tensor("blkseg", [P, P], f32, kind="ExternalInput")
    sel8_d = nc.dram_tensor("sel8", [P, SEG], f32, kind="ExternalInput")
    dmask_d = nc.dram_tensor("dmask", [P, JC * D], f32, kind="ExternalInput")
    out_d = nc.dram_tensor("out", [SEG, D], f32, kind="ExternalOutput")

    pieces_per_group = _redist_pieces()

    with tile.TileContext(nc) as tc:
        with (
            tc.tile_pool(name="persist", bufs=1) as pp,
            tc.tile_pool(name="xina", bufs=2) as xina,
            tc.tile_pool(name="stage", bufs=3) as stg,
            tc.tile_pool(name="xinc", bufs=3) as xinc,
            tc.tile_pool(name="lhsp", bufs=4) as lhsp,
        ):
            wq = pp.tile([P, 4], bf16)
            blkb = pp.tile([P, SEG], bf16)
            blkf = pp.tile([P, SEG], f32)
            blkseg = pp.tile([P, P], f32)
            sel8 = pp.tile([P, SEG], f32)
            dmask = pp.tile([P, JC * D], f32)
            att = pp.tile([P, JP], f32)
            mask = pp.tile([P, JP], bf16)
            nc.sync.dma_start(out=wq, in_=wq_d[:, :])
            nc.sync.dma_start(out=blkb, in_=blkb_d[:, :])
            nc.sync.dma_start(out=blkf, in_=blkf_d[:, :])
            nc.sync.dma_start(out=blkseg, in_=blkseg_d[:, :])
            nc.sync.dma_start(out=sel8, in_=sel8_d[:, :])
            nc.sync.dma_start(out=dmask, in_=dmask_d[:, :])

            # warm-up: absorb const-DMA waits
            warm = pp.tile([P, 1], f32)
            nc.vector.tensor_copy(out=warm, in_=blkseg[:, 0:1])
            nc.vector.tensor_copy(out=warm, in_=blkf[:, 0:1])
            warmb = pp.tile([P, 1], bf16)
            nc.vector.tensor_copy(out=warmb, in_=wq[:, 0:1])
            nc.vector.tensor_copy(out=warmb, in_=blkb[:, 0:1])

            # ================= Phase A: att via PE =================
            def copy_any(which, out, in_):
                if which == 0:
                    nc.vector.tensor_copy(out=out, in_=in_)
                elif which == 1:
                    nc.scalar.activation(out=out, in_=in_,
                                         func=Act.Copy)
                else:
                    nc.gpsimd.tensor_copy(out=out, in_=in_)

            copy_rot = [0, 1, 0, 0, 1, 0]          # V,A,V,V,A,V (no gpsimd: PSUM)
            dma_rot = [nc.sync, nc.sync, nc.scalar, nc.sync, nc.gpsimd,
                       nc.scalar]
            with tc.tile_pool(name="psa", bufs=2, space="PSUM") as psa:
                ndma = 0
                for t in range(NST_A):
                    xtile = xina.tile([P, ST_A], bf16, tag="xa")
                    src = bass.AP(tensor=xt_d, offset=t * ST_A,
                                  ap=[[NCOL, P], [1, ST_A]])
                    nc.sync.dma_start(out=xtile, in_=src)
                    for gl in range(7):
                        g = t * 7 + gl
                        ps = psa.tile([4, CH_A * GRP_A], f32, tag="aps")
                        for c in range(GRP_A):
                            rhs = xtile[:, (gl * GRP_A + c) * CH_A:
                                        (gl * GRP_A + c + 1) * CH_A]
                            nc.tensor.matmul(out=ps[:, c * CH_A:(c + 1) * CH_A],
                                             lhsT=wq, rhs=rhs,
                                             start=True, stop=True)
                        sbuf_stage = stg.tile([4, CH_A * GRP_A], f32, tag="stg")
                        copy_any(copy_rot[g % 6], sbuf_stage, ps)
                        for (soff, pstart, j0, ln) in pieces_per_group[g]:
                            src_ap = sbuf_stage[0:4, soff:soff + ln]
                            dst_ap = att[pstart::32, j0:j0 + ln]
                            dma_rot[ndma % 6].dma_start(out=dst_ap, in_=src_ap)
                            ndma += 1

            # ================= Phase B: bisection =================
            with tc.tile_pool(name="psb", bufs=2, space="PSUM") as psb:
                pmin = pp.tile([P, 1], f32)
                pmax = pp.tile([P, 1], f32)
                lo = pp.tile([P, 1], f32)
                hi = pp.tile([P, 1], f32)
                mid = pp.tile([P, 1], f32)
                tmp = pp.tile([P, 1], f32)
                cnt = pp.tile([P, 1], f32)
                g1 = pp.tile([P, 1], mybir.dt.int32)
                g2 = pp.tile([P, 1], mybir.dt.int32)
                onesp = pp.tile([P, 1], f32)
                nc.vector.memset(onesp, 1.0)
                ones_b = bass.AP(tensor=onesp.tensor, offset=onesp.offset,
                                 ap=[onesp.ap[0], [0, JP]])

                nc.vector.tensor_reduce(out=pmax, in_=att,
                                        axis=mybir.AxisListType.X, op=Alu.max)
                nc.vector.tensor_reduce(out=pmin, in_=att,
                                        axis=mybir.AxisListType.X, op=Alu.min)
                nc.vector.tensor_scalar(out=pmax, in0=pmax, scalar1=0.0,
                                        scalar2=None, op0=Alu.max)
                nc.vector.tensor_scalar(out=pmin, in0=pmin, scalar1=0.0,
                                        scalar2=None, op0=Alu.min)
                lo_ps = psb.tile([P, 1], f32, tag="mmb")
                nc.tensor.matmul(out=lo_ps, lhsT=blkseg, rhs=pmin,
                                 start=True, stop=True)
                nc.vector.tensor_scalar(out=lo, in0=lo_ps, scalar1=1.0,
                                        scalar2=None, op0=Alu.subtract)
                hi_ps = psb.tile([P, 1], f32, tag="mmb")
                nc.tensor.matmul(out=hi_ps, lhsT=blkseg, rhs=pmax,
                                 start=True, stop=True)
                nc.vector.tensor_copy(out=hi, in_=hi_ps)

                for it in range(NITER + 1):
                    nc.vector.tensor_tensor(out=tmp, in0=lo, in1=hi, op=Alu.add)
                    nc.vector.tensor_scalar(out=mid, in0=tmp, scalar1=0.5,
                                            scalar2=None, op0=Alu.mult)
                    nc.vector.scalar_tensor_tensor(
                        out=mask, in0=att, scalar=mid[:, :], in1=ones_b,
                        op0=Alu.is_gt, op1=Alu.mult, accum_out=cnt)
                    if it == NITER:
                        break
                    cnt_ps = psb.tile([P, 1], f32, tag="mmb")
                    nc.tensor.matmul(out=cnt_ps, lhsT=blkseg, rhs=cnt,
                                     start=True, stop=True)
                    nc.vector.tensor_scalar(out=g1, in0=cnt_ps,
                                            scalar1=float(TOPK), scalar2=None,
                                            op0=Alu.is_ge)
                    nc.vector.tensor_scalar(out=g2, in0=cnt_ps,
                                            scalar1=float(TOPK), scalar2=None,
                                            op0=Alu.is_lt)
                    nc.vector.copy_predicated(out=lo, mask=g1, data=mid)
                    nc.vector.copy_predicated(out=hi, mask=g2, data=mid)

                # ================= Phase C: masked sum =================
                with tc.tile_pool(name="psc", bufs=1, space="PSUM") as psc:
                    diag_ps = psc.tile([P, JC * D], f32)
                    acc = pp.tile([P, D], f32)
                    first_dense = True
                    lhs_engines = [nc.vector, nc.gpsimd]
                    blkb_b = bass.AP(tensor=blkb.tensor, offset=blkb.offset,
                                     ap=[blkb.ap[0], [0, JC], [1, SEG]])
                    n_mm = 0
                    total_mm = NST_C * NPE_C
                    for t in range(NST_C):
                        xtile = xinc.tile([P, JT_C, D], bf16, tag="xc")
                        src = bass.AP(tensor=xo_d, offset=t * JT_C * D,
                                      ap=[[NCOL, P], [1, JT_C * D]])
                        nc.sync.dma_start(out=xtile, in_=src)
                        jbase = t * JT_C
                        # PE path: chunks [0, NPE_C*JC)
                        for c in range(NPE_C):
                            j0 = jbase + c * JC
                            lhsT = lhsp.tile([P, JC, SEG], bf16, tag="lhs")
                            msk_b = bass.AP(
                                tensor=mask.tensor, offset=mask.offset + j0,
                                ap=[mask.ap[0], [1, JC], [0, SEG]])
                            lhs_engines[c % 2].tensor_tensor(
                                out=lhsT, in0=msk_b, in1=blkb_b, op=Alu.mult)
                            rhs = xtile[:, c * JC:(c + 1) * JC, :]
                            nc.tensor.matmul(out=diag_ps, lhsT=lhsT, rhs=rhs,
                                             start=(n_mm == 0),
                                             stop=(n_mm == total_mm - 1))
                            n_mm += 1
                        # dense path: j in [NPE_C*JC, JT_C)
                        dslice = xtile[:, NPE_C * JC:JT_C, :]
                        mask_b3 = bass.AP(
                            tensor=mask.tensor,
                            offset=mask.offset + jbase + NPE_C * JC,
                            ap=[mask.ap[0], [1, JDENSE], [0, D]])
                        nc.gpsimd.tensor_tensor(out=dslice, in0=dslice,
                                                in1=mask_b3, op=Alu.mult)
                        dT = bass.AP(
                            tensor=xtile.tensor,
                            offset=xtile.offset + (NPE_C * JC) * D,
                            ap=[xtile.ap[0], [1, D], [D, JDENSE]])
                        pt = pp.tile([P, D], f32, tag=f"pt{t}")
                        nc.vector.tensor_reduce(out=pt, in_=dT,
                                                axis=mybir.AxisListType.X,
                                                op=Alu.add)
                        if first_dense:
                            nc.vector.tensor_copy(out=acc, in_=pt)
                            first_dense = False
                        else:
                            nc.vector.tensor_tensor(out=acc, in0=acc, in1=pt,
                                                    op=Alu.add)

                    # ---- collect: mask off-diag, gather via matmul ----
                    acc8 = pp.tile([SEG, D], f32)
                    diag_sb = pp.tile([P, JC * D], f32)
                    nc.vector.tensor_tensor(out=diag_sb, in0=diag_ps,
                                            in1=dmask, op=Alu.mult)
                    mm2_ps = psb.tile([SEG, JC * D], f32, tag="mm2")
                    nc.tensor.matmul(out=mm2_ps, lhsT=sel8, rhs=diag_sb,
                                     start=True, stop=True)
                    red_ap = bass.AP(tensor=mm2_ps.tensor,
                                     offset=mm2_ps.offset,
                                     ap=[mm2_ps.ap[0], [1, D], [D, JC]])
                    nc.vector.tensor_reduce(out=acc8, in_=red_ap,
                                            axis=mybir.AxisListType.X,
                                            op=Alu.add)
                    dsum_ps = psb.tile([SEG, D], f32, tag="dsum")
                    nc.tensor.matmul(out=dsum_ps, lhsT=blkf, rhs=acc,
                                     start=True, stop=True)
                    nc.vector.tensor_tensor(out=acc8, in0=acc8, in1=dsum_ps,
                                            op=Alu.add)

                    # ---- normalize ----
                    sq = pp.tile([SEG, D], f32)
                    nrm2 = pp.tile([SEG, 1], f32)
                    nrm = pp.tile([SEG, 1], f32)
                    rinv = pp.tile([SEG, 1], f32)
                    outt = pp.tile([SEG, D], f32)
                    nc.vector.scalar_tensor_tensor(
                        out=sq, in0=acc8, scalar=1.0, in1=acc8,
                        op0=Alu.mult, op1=Alu.mult, accum_out=nrm2)
                    nc.scalar.activation(out=nrm, in_=nrm2, func=Act.Sqrt)
                    nc.vector.tensor_scalar(out=nrm, in0=nrm, scalar1=1e-12,
                                            scalar2=None, op0=Alu.max)
                    nc.vector.reciprocal(out=rinv, in_=nrm)
                    nc.vector.tensor_scalar(out=outt, in0=acc8,
                                            scalar1=rinv[:, :], scalar2=None,
                                            op0=Alu.mult)
                    nc.sync.dma_start(out=out_d[:, :], in_=outt)

    _hoist_sync_waits(nc)
    return nc


def _constants():
    import ml_dtypes
    blk = np.zeros((P, SEG), np.float32)
    for p in range(P):
        blk[p, p // 16] = 1.0
    blkseg = np.zeros((P, P), np.float32)
    for p in range(P):
        blkseg[p, (p // 16) * 16:(p // 16 + 1) * 16] = 1.0
    sel8 = np.zeros((P, SEG), np.float32)
    for p in range(P):
        sel8[p, p % SEG] = 1.0
    dmask = np.zeros((P, JC * D), np.float32)
    for p in range(P):
        dmask[p, (p // SEG) * D:(p // SEG + 1) * D] = 1.0
    return blk.astype(ml_dtypes.bfloat16), blk, blkseg, sel8, dmask


def _pack_core(xc, w):
    """xc: [400000, 32] f32 -> (xt, xo) [128, 100352] bf16."""
    import ml_dtypes
    xp = np.zeros((SEG, LP, D), np.float32)
    xp[:, :L, :] = xc.reshape(SEG, L, D)
    xo = np.ascontiguousarray(
        xp.reshape(SEG, 16, JP, D).reshape(P, NCOL)).astype(ml_dtypes.bfloat16)
    xq = xp.reshape(4, NPQ, D)
    xt = np.ascontiguousarray(
        xq.transpose(0, 2, 1).reshape(P, NCOL)).astype(ml_dtypes.bfloat16)
    return xt, xo


def kernel(x, length, w, b):
    import ml_dtypes
    from concourse.bass_utils import run_bass_kernel_spmd

    x = np.asarray(x, dtype=np.float32)
    w = np.asarray(w, dtype=np.float32)

    if "nc" not in _CACHE:
        _CACHE["nc"] = _build()
        _CACHE["consts"] = _constants()
    nc = _CACHE["nc"]
    blkb, blkf, blkseg, sel8, dmask = _CACHE["consts"]

    wq = np.zeros((P, 4), np.float32)
    for q in range(4):
        wq[q * 32:(q + 1) * 32, q] = w
    wq = wq.astype(ml_dtypes.bfloat16)

    NROW = SEG * L
    in_maps = []
    for i in range(NCORES):
        xt, xo = _pack_core(x[i * NROW:(i + 1) * NROW], w)
        in_maps.append({"xt": xt, "xo": xo, "wq": wq, "blkb": blkb,
                        "blkf": blkf, "blkseg": blkseg, "sel8": sel8,
                        "dmask": dmask})

    r = run_bass_kernel_spmd(nc, in_maps, list(range(NCORES)))
    out = np.concatenate([r.results[i]["out"] for i in range(NCORES)], axis=0)
    return out.astype(np.float32)
